# revision 3
# baseline (speedup 1.0000x reference)
"""Trainium2 Bass kernel for nn_CustomMultiLossLayer (heteroscedastic MC classification loss).

Math (per head h):
  d[t,n,c]  = logits[n,c] + eps[t,n,c]*scale[n],  scale = exp(0.5*y_pred[:,3])
  LSE[t,n]  = log(sum_c exp(d))
  ce[t,n]   = w[n]*LSE[t,n] - sum_c y[n,c]*d[t,n,c],  w[n] = sum_c y[n,c]
  mc_h      = mean_{t,n} ce
  loss      = sum_h exp(-lv_h)*mc_h + lv_h

Device computes (data-parallel over N across 8 cores, shard = 4096 rows):
  A[n]   = sum_t [ln(2^-24 * sum_c exp(scale[n]*eps + logit[n,c]))]   (= sum_t LSE - T*24*ln2)
  R[n,c] = sum_t eps[t,n,c]
Host folds the small tensors (y_true, y_pred, log_vars) in float64.

Layout: host permutes each eps shard to [T, C, N_SH] (c-major). On-chip:
  - DMA (SWDGE, f32->bf16 cast) loads X tiles [125t, 1536(c,n)]
  - PE transposes 128-col windows into PSUM [128n, 500t] (bf16)
  - ACT: exp(scale_il*P + bias_il) with per-partition affine -> E bf16
  - DVE: s = E0+E1+E2 ; ACT: Ln(2^-24 * s) with accum_out -> A column
  - PE ones-matmuls accumulate R in PSUM; DVE copies out.
"""

import os
import numpy as np
import ml_dtypes

import concourse.bacc as bacc
import concourse.tile as tile
from concourse import mybir
from concourse.bass_utils import run_bass_kernel_spmd

# Problem constants (hardcoded per harness contract)
T = 500
C = 3
N = 32768
NCORES = 8
NSH = N // NCORES            # 4096 rows per core
TCH = 125                    # t-chunk rows (500 = 4*125)
NTC = 4
SLICE = 512                  # n's per slice
NSLICES = NSH // SLICE       # 8
NV = 4                       # 128-wide n windows per slice
SHIFT = 24                   # Ln input scaled by 2^-SHIFT (ACT Ln valid range)
LN2 = float(np.log(2.0))

_CACHE = {}
LAST_RESULTS = None


def _build_nc():
    f32 = mybir.dt.float32
    bf16 = mybir.dt.bfloat16
    Exp = mybir.ActivationFunctionType.Exp
    Ln = mybir.ActivationFunctionType.Ln

    nc = bacc.Bacc()
    eps_d = [
        nc.dram_tensor("eps_cn0", [T, C * SLICE * NSLICES], f32, kind="ExternalInput"),
        nc.dram_tensor("eps_cn1", [T, C * SLICE * NSLICES], f32, kind="ExternalInput"),
    ]
    scale_d = nc.dram_tensor("scale_t", [2, NSLICES, 128, NV], f32, kind="ExternalInput")
    bias_d = nc.dram_tensor("bias_t", [2, NSLICES, 128, NV * C], f32, kind="ExternalInput")
    ident_d = nc.dram_tensor("ident", [TCH, TCH], bf16, kind="ExternalInput")
    ones_d = nc.dram_tensor("ones_col", [TCH, 1], bf16, kind="ExternalInput")
    a_d = nc.dram_tensor("A_out", [128, 2 * NSLICES * NV], f32, kind="ExternalOutput")
    r_d = nc.dram_tensor("R_out", [2 * NSLICES, C * SLICE], f32, kind="ExternalOutput")

    with tile.TileContext(nc) as tc:
        with (
            tc.tile_pool(name="consts", bufs=1) as cpool,
            tc.tile_pool(name="xpool", bufs=12) as xpool,
            tc.tile_pool(name="epool", bufs=8) as epool,
            tc.tile_pool(name="spool", bufs=4) as spool,
            tc.tile_pool(name="mpool", bufs=3) as mpool,
            tc.tile_pool(name="apool", bufs=1) as apool,
            tc.tile_pool(name="ppool", bufs=4, space="PSUM") as ppool,
            tc.tile_pool(name="rpool", bufs=1, space="PSUM") as rpool,
        ):
            ident = cpool.tile([TCH, TCH], bf16)
            nc.sync.dma_start(ident, ident_d[:, :])
            ones_col = cpool.tile([TCH, 1], bf16)
            nc.sync.dma_start(ones_col, ones_d[:, :])
            a_sb = apool.tile([128, 2 * NSLICES * NV], f32)

            for h in range(2):
                for sl in range(NSLICES):
                    scale_sl = mpool.tile([128, NV], f32, tag="scale")
                    nc.sync.dma_start(scale_sl, scale_d[h, sl])
                    bias_sl = mpool.tile([128, NV * C], f32, tag="bias")
                    nc.sync.dma_start(bias_sl, bias_d[h, sl])

                    r_ps = rpool.tile([1, C * SLICE], f32, tag="rps")
                    xs = []
                    for tcn in range(NTC):
                        x_t = xpool.tile([TCH, C * SLICE], bf16, tag="X",
                                         name=f"X_{h}_{sl}_{tcn}")
                        nc.gpsimd.dma_start(
                            x_t,
                            eps_d[h][TCH * tcn: TCH * (tcn + 1),
                                     C * SLICE * sl: C * SLICE * (sl + 1)],
                        )
                        xs.append(x_t)
                        for k in range(C):
                            nc.tensor.matmul(
                                r_ps[:, 512 * k: 512 * (k + 1)],
                                ones_col[:, :],
                                x_t[:, 512 * k: 512 * (k + 1)],
                                start=(tcn == 0),
                                stop=(tcn == NTC - 1),
                            )
                    r_sb = mpool.tile([1, C * SLICE], f32, tag="rsb")
                    nc.vector.tensor_copy(r_sb, r_ps)
                    nc.sync.dma_start(r_d[h * NSLICES + sl: h * NSLICES + sl + 1, :], r_sb)

                    for v in range(NV):
                        es = []
                        for c in range(C):
                            # t-chunk regions padded to 128 cols so bf16 PSUM
                            # writes stay 4B-aligned; exp skips the gap cols
                            # via a 3D access pattern.
                            p_ps = ppool.tile([128, 128 * NTC], bf16, tag="P",
                                              name=f"P_{h}_{sl}_{v}_{c}")
                            for tcn in range(NTC):
                                nc.tensor.transpose(
                                    p_ps[:, 128 * tcn: 128 * tcn + TCH],
                                    xs[tcn][:, SLICE * c + 128 * v: SLICE * c + 128 * (v + 1)],
                                    ident[:, :],
                                )
                            e_t = epool.tile([128, T], bf16, tag="E",
                                             name=f"E_{h}_{sl}_{v}_{c}")
                            p3 = p_ps.rearrange("p (k t) -> p k t", k=NTC)[:, :, 0:TCH]
                            e3 = e_t.rearrange("p (k t) -> p k t", k=NTC)
                            nc.scalar.activation(
                                e3, p3, Exp,
                                bias=bias_sl[:, C * v + c: C * v + c + 1],
                                scale=scale_sl[:, v: v + 1],
                            )
                            es.append(e_t)
                        s_t = spool.tile([128, T], bf16, tag="s", name=f"s_{h}_{sl}_{v}")
                        nc.vector.tensor_add(s_t, es[0], es[1])
                        nc.vector.tensor_add(s_t, s_t, es[2])
                        l_scr = spool.tile([128, T], bf16, tag="lscr", name=f"L_{h}_{sl}_{v}")
                        col = (NSLICES * NV) * h + NV * sl + v
                        nc.scalar.activation(
                            l_scr, s_t, Ln,
                            scale=float(2.0 ** -SHIFT),
                            accum_out=a_sb[:, col: col + 1],
                        )
            nc.sync.dma_start(a_d[:, :], a_sb)
    nc.compile()
    return nc


def kernel(**inputs):
    global LAST_RESULTS
    y_true = [np.asarray(inputs["y_true0"], dtype=np.float64),
              np.asarray(inputs["y_true1"], dtype=np.float64)]
    y_pred = [np.asarray(inputs["y_pred0"], dtype=np.float64),
              np.asarray(inputs["y_pred1"], dtype=np.float64)]
    log_vars = np.asarray(inputs["log_vars"], dtype=np.float64)
    eps = [np.asarray(inputs["eps0"], dtype=np.float32),
           np.asarray(inputs["eps1"], dtype=np.float32)]

    if "nc" not in _CACHE:
        _CACHE["nc"] = _build_nc()
    nc = _CACHE["nc"]

    # ---- host prep -------------------------------------------------------
    # eps [T, N, C] -> per-core [T, NSLICES, C, SLICE] (c-major within each
    # 512-n slice), flattened [T, C*NSH]: col = sl*1536 + c*512 + n_in_slice
    eps_cn = [
        np.ascontiguousarray(
            e.reshape(T, NCORES, NSLICES, SLICE, C).transpose(1, 0, 2, 4, 3)
        ).reshape(NCORES, T, C * NSH)
        for e in eps
    ]
    scale = np.stack([np.exp(0.5 * yp[:, C]) for yp in y_pred])       # [2, N] f64
    logits = np.stack([yp[:, :C] for yp in y_pred])                   # [2, N, C]

    # scale_t[core]: [2, NSLICES, 128, NV];  n = core*NSH + sl*512 + v*128 + p
    sc_t = (scale.reshape(2, NCORES, NSLICES, NV, 128)
                 .transpose(1, 0, 2, 4, 3).astype(np.float32))        # [core,2,8,128,4]
    bi_t = (logits.reshape(2, NCORES, NSLICES, NV, 128, C)
                  .transpose(1, 0, 2, 4, 3, 5)
                  .reshape(NCORES, 2, NSLICES, 128, NV * C).astype(np.float32))

    ident = np.eye(TCH, dtype=ml_dtypes.bfloat16)
    ones_col = np.ones((TCH, 1), dtype=ml_dtypes.bfloat16)

    in_maps = []
    for core in range(NCORES):
        in_maps.append({
            "eps_cn0": eps_cn[0][core],
            "eps_cn1": eps_cn[1][core],
            "scale_t": np.ascontiguousarray(sc_t[core]),
            "bias_t": np.ascontiguousarray(bi_t[core]),
            "ident": ident,
            "ones_col": ones_col,
        })

    trace = bool(int(os.environ.get("KERNEL_TRACE", "0")))
    res = run_bass_kernel_spmd(nc, in_maps, core_ids=list(range(NCORES)),
                               trace=trace)
    LAST_RESULTS = res

    # ---- host combine (float64) -----------------------------------------
    A = np.stack([r["A_out"] for r in res.results]).astype(np.float64)   # [8,128,64]
    R = np.stack([r["R_out"] for r in res.results]).astype(np.float64)   # [8,16,1536]

    # A[core][p, 32h+4sl+v] -> [2, N];  n = core*NSH + sl*512 + v*128 + p
    A_n = (A.reshape(NCORES, 128, 2, NSLICES, NV)
            .transpose(2, 0, 3, 4, 1).reshape(2, N))
    sum_lse = A_n + T * SHIFT * LN2                                      # sum_t LSE per n

    # R[core][h*8+sl, c*512+j] -> [2, N, C]
    R_n = (R.reshape(NCORES, 2, NSLICES, C, SLICE)
            .transpose(1, 0, 2, 4, 3).reshape(2, N, C))

    loss = 0.0
    for h in range(2):
        w = y_true[h].sum(axis=1)                                        # [N]
        term1 = float(np.dot(w, sum_lse[h]))
        term2 = T * float(np.sum(y_true[h] * logits[h])) + \
            float(np.sum(y_true[h] * scale[h][:, None] * R_n[h]))
        mc = (term1 - term2) / (T * N)
        loss += np.exp(-log_vars[h]) * mc + log_vars[h]
    return np.asarray(loss, dtype=np.float32)
